# revision 29
# baseline (speedup 1.0000x reference)
"""Trainium2 Bass kernel for the se3ACN encoder (gnn_message_passing).

Strategy (final: radial-MLP tabulation, J=16 nodes, 8 source atoms per matmul)
------------------------------------------------------------------------------
The per-pair radial MLP (3 -> 150 -> 150 -> 150 -> Cout*Cin, softplus) depends
only on the scalar pair distance r.  Tabulate K_c(r) = MLP_c(r)/sqrt(cin) on
J=16 piecewise-linear hats  hat_j(v) = relu(1 - |v - j|),  v = r/DELTA, with
node values least-squares fitted on a fine grid (end-to-end rel err ~1.9e-3
on hardware vs the 2e-2 gate).  The neighbor cutoff is exact: masked pairs
(r >= 3) get v shifted by +4 so every hat is exactly 0.

Per cloud the message passing becomes

    feat'[n,o] = sum_m sum_j hat[j,(m,n)] * G[m,j,o],
    G[m,j,o]   = sum_i T_c[j,o,i] * feat[m,i].

With J=16, EIGHT source atoms (group g: m = 8g+s) stack on the 128 matmul
partitions (rows q = 16*s + j), so each K=128 fp16 matmul accumulates eight
atoms' messages: 36 matmuls per cloud.  The atom count is padded 286 -> 288
with two far-away (masked) dummy atoms.

hat generation per group: a broadcast SBUF->SBUF DMA (stride-0 src AP,
issued alternately from the idle gpsimd/sync queues) replicates the eight
v rows over the eight 16-partition blocks -- no PE work; then ONE ACT Abs
(per-partition bias -(q%16)) and ONE DVE tensor_scalar (min(a,1)-1 = -hat,
sign folded into the tables) cover ABATCH groups per instruction (engine
cost is free-dim bound).  hneg stays SBUF-resident ([128, 36*286] fp16,
~21KB/partition) and serves all 3 clouds; cloud-0 accumulation is fused
into the generation loop.

Geometry: r^2 via one 5-row matmul per 96-atom chunk (the -2x.y + |x|^2 +
|y|^2 trick), then v = r/DELTA with an Abs_reciprocal_sqrt table + one
Newton step (table error squared away).  Per cloud, features are regrouped
into RF[(s,i), g] = feat[8g+s, i] by 8 accumulating matmuls with block
one-hot stationaries, then 8 table matmuls -> G (one fused psum->fp16
copy).  All ACT functions used (Abs_reciprocal_sqrt, Abs, Square, Copy)
live in the single 'abs_reciprocal_sqrt_and_small' table set -> one table
load total.

Sharding: cores (2b, 2b+1) both compute molecule b (redundant pair; the
even core's result is read back).  The 4x24 head (batch-coupled batchnorm
over the 4 molecules) runs on host.  ~72us on hardware vs the 6113us
baseline (~85x).
"""

import math

import numpy as np

import concourse.bass as bass
import concourse.mybir as mybir
import concourse.tile as tile
from concourse import bacc
from concourse.bass_utils import run_bass_kernel_spmd

AF = mybir.ActivationFunctionType
ALU = mybir.AluOpType
F32 = mybir.dt.float32
F32R = mybir.dt.float32r
F16 = mybir.dt.float16

B, N = 4, 286
EMB, CD, NCLOUD = 4, 8, 3
H = 150
BETA = 5.0
RADII = (0.0, 1.5, 3.0)
RSTEP = 1.5
MAXR = 3.0
NCORES = 8
J = 16                       # tabulation nodes
DELTA = MAXR / (J - 1)
VSHIFT = 4.0                 # pushes masked pairs out of every hat support
SPG = 8                      # source atoms per group
NP = 288                     # padded atom count (2 masked dummies)
NG = NP // SPG               # 72 groups
QP = SPG * J                 # 128 hat partitions
ABATCH = 4                   # phase-A groups emitted per stage
GEO_CHUNKS = ((0, 96), (96, 96), (192, 96))
HN = N // 2                  # last-cloud n-columns per core of a pair


class _PackLayout:
    """Column layouts of the packed constant tensors ([128, cols])."""

    def __init__(self):
        # float16 pack (tables + regroup + initial features)
        o = 0
        self.wg = []                                # per cloud: [SPG*cin, CD*QP]
        for c in range(NCLOUD):
            self.wg.append(o); o += CD * QP
        self.rsel = o; o += SPG * SPG * CD          # [CD, SPG*CD] per s block
        self.rf0 = o; o += NG                       # [SPG*EMB, NG] cloud-0 RF
        self.cols_h = o
        # float32 pack (geometry + abs bias)
        o = 0
        self.geomA = o; o += NP
        self.geomB = o; o += N
        self.absb = o; o += 1                       # [96, 1] = -(q % 32)
        self.cols_f = o


def _build(nc):
    L = _PackLayout()

    packh = nc.declare_dram_parameter("packh", [96, L.cols_h], F16, isOutput=False)
    packf = nc.declare_dram_parameter("packf", [128, L.cols_f], F32, isOutput=False)
    sumsq = nc.declare_dram_parameter("sumsq", [CD, NCLOUD], F32, isOutput=True)

    with tile.TileContext(nc) as tc:
        with (
            tc.tile_pool(name="const", bufs=1) as cp,
            tc.tile_pool(name="abuf", bufs=2) as ab,
            tc.tile_pool(name="gbuf", bufs=2) as gp,
            tc.tile_pool(name="ft", bufs=2) as ftp,
            tc.tile_pool(name="misc", bufs=2) as mp,
            tc.tile_pool(name="vrep", bufs=4) as vrp,
            tc.tile_pool(name="pa", bufs=2, space=bass.MemorySpace.PSUM) as pa,
            tc.tile_pool(name="pg", bufs=1, space=bass.MemorySpace.PSUM) as pg,
            tc.tile_pool(name="prf", bufs=1, space=bass.MemorySpace.PSUM) as prf,
            tc.tile_pool(name="pacc", bufs=2, space=bass.MemorySpace.PSUM) as pacc,
        ):
            pf = cp.tile([128, L.cols_f], F32, tag="packf")
            nc.sync.dma_start(out=pf[:], in_=packf[:])
            ph = cp.tile([96, L.cols_h], F16, tag="packh")
            # split the pack DMA over column slices (parallel queues), and
            # issue from gpsimd whose DMA issue cost is ~25ns vs sync ~600ns
            NSL = 8
            slw = -(-L.cols_h // NSL)
            for si in range(NSL):
                c0, c1 = si * slw, min((si + 1) * slw, L.cols_h)
                if c0 < c1:
                    nc.gpsimd.dma_start(out=ph[:, c0:c1], in_=packh[:, c0:c1])

            geomA_sb = pf[0:5, L.geomA:L.geomA + NP]
            geomB_sb = pf[0:5, L.geomB:L.geomB + N]
            absb_sb = pf[0:QP, L.absb:L.absb + 1]
            out_sb = cp.tile([CD, NCLOUD], F32, tag="out")

            # big SBUF-resident -hat matrix: [32*s + j, g*N + n] fp16
            hneg = cp.tile([QP, NG * N], F16, tag="hneg")

            # ---- geometry: v = r/DELTA (+4 where r >= MAXR), [m-chunk, N]
            v_tiles = []
            for ci, (off, pm) in enumerate(GEO_CHUNKS):
                r2p = pa.tile([128, N], F32, tag="pa")
                nc.tensor.matmul(
                    r2p[0:pm, :], geomA_sb[:, off:off + pm], geomB_sb,
                    start=True, stop=True,
                )
                r2c = mp.tile([128, N], F32, tag="r2c")
                nc.vector.tensor_scalar_max(r2c[0:pm, :], r2p[0:pm, :], 1e-12)
                # v = r/DELTA via rsqrt table + one Newton step:
                # y1 = y0*(1.5 - 0.5*r2*y0^2), r = r2*y1 (table err squared)
                y0 = mp.tile([128, N], F32, tag="y0")
                nc.scalar.activation(y0[0:pm, :], r2c[0:pm, :],
                                     AF.Abs_reciprocal_sqrt)
                y2 = mp.tile([128, N], F32, tag="y2")
                nc.scalar.activation(y2[0:pm, :], y0[0:pm, :], AF.Square)
                h2 = mp.tile([128, N], F32, tag="h2")
                nc.vector.tensor_mul(h2[0:pm, :], r2c[0:pm, :], y2[0:pm, :])
                cf = mp.tile([128, N], F32, tag="cf")
                nc.vector.tensor_scalar(
                    out=cf[0:pm, :], in0=h2[0:pm, :],
                    scalar1=float(-0.5 / DELTA), scalar2=float(1.5 / DELTA),
                    op0=ALU.mult, op1=ALU.add,
                )
                y1 = mp.tile([128, N], F32, tag="y1")
                nc.vector.tensor_mul(y1[0:pm, :], y0[0:pm, :], cf[0:pm, :])
                vt = mp.tile([128, N], F32, tag="vt")
                nc.vector.tensor_mul(vt[0:pm, :], r2c[0:pm, :], y1[0:pm, :])
                sh = mp.tile([128, N], F32, tag="sh")
                nc.vector.tensor_scalar(
                    out=sh[0:pm, :], in0=vt[0:pm, :],
                    scalar1=float(J - 1), scalar2=VSHIFT,
                    op0=ALU.is_ge, op1=ALU.mult,
                )
                vch = cp.tile([128, N], F16, tag=f"v_{ci}")
                nc.vector.tensor_add(vch[0:pm, :], vt[0:pm, :], sh[0:pm, :])
                v_tiles.append(vch)

            def emit_G(c, rf_sb, kin):
                """G[J*s+j, o*NG+g] = -sum_i T_c[j,o,i] feat[SPG*g+s,i]."""
                G = gp.tile([QP, CD * NG], F16, tag="G")
                gt = pg.tile([QP, CD * NG], F32, tag="pg")
                for o in range(CD):
                    wg_o = ph[0:kin, L.wg[c] + o * QP:L.wg[c] + (o + 1) * QP]
                    nc.tensor.matmul(gt[:, o * NG:(o + 1) * NG], wg_o, rf_sb,
                                     start=True, stop=True)
                nc.scalar.copy(G[:, :], gt[:, :])
                return G

            rf0_sb = ph[0:SPG * EMB, L.rf0:L.rf0 + NG]
            G0 = emit_G(0, rf0_sb, SPG * EMB)

            # ---- phase A: hat generation fused with cloud-0 accumulation,
            # emitted in batches so the PE sees long gapless matmul stretches
            acc0 = pacc.tile([CD, N], F32, tag="acc")
            dma_eng = [nc.gpsimd, nc.sync]
            batches = []
            g0 = 0
            for w in (1, 1, 2):
                batches.append(list(range(g0, g0 + w))); g0 += w
            while g0 < NG:
                batches.append(list(range(g0, min(g0 + ABATCH, NG))))
                g0 += ABATCH
            dma_eng3 = [nc.gpsimd, nc.sync, nc.scalar]
            for bi, gs in enumerate(batches):
                g0 = gs[0]
                nb = len(gs)
                late = bi >= 3
                engs = dma_eng3 if late else dma_eng
                # one v_rep tile holds ABATCH groups side by side; each group
                # filled by a broadcast SBUF->SBUF DMA (v rows replicated over
                # the 8 16-partition blocks).  Early batches: issues on
                # gpsimd/sync, |v-j| on ACT (DVE busy with geometry).  Later
                # batches: ACT joins the issue rotation and DVE does the abs
                # (one fp16 2x-mode tensor_scalar per batch).
                vr = vrp.tile([128, ABATCH * N], F16, tag="vr")
                for k, g in enumerate(gs):
                    ci = (SPG * g) // 96
                    off, pm = GEO_CHUNKS[ci]
                    c0 = SPG * g - off
                    engs[(bi + k) % len(engs)].dma_start(
                        out=vr[:, k * N:(k + 1) * N],
                        in_=v_tiles[ci][c0:c0 + SPG, :]
                        .unsqueeze(1).broadcast_to([SPG, J, N]),
                    )
                a = ab.tile([QP, ABATCH * N], F16, tag="a")
                nc.scalar.activation(a[:, 0:nb * N], vr[0:QP, 0:nb * N],
                                     AF.Abs, bias=absb_sb)
                nc.vector.tensor_scalar(
                    out=hneg[:, g0 * N:(g0 + nb) * N], in0=a[:, 0:nb * N],
                    scalar1=1.0, scalar2=1.0,
                    op0=ALU.min, op1=ALU.subtract,
                )
                for g in gs:
                    nc.tensor.matmul(
                        acc0[:, :], G0[:, g:CD * NG:NG],
                        hneg[:, g * N:(g + 1) * N],
                        start=(g == 0), stop=(g == NG - 1),
                    )

            # ---- cloud epilogues + clouds 1, 2
            sq = mp.tile([CD, N], F32, tag="sq")
            nc.scalar.activation(sq[:, :], acc0[:, :], AF.Square,
                                 accum_out=out_sb[:, 0:1])

            acc_prev = acc0
            for c in range(1, NCLOUD):
                # ft [8, 288] fp16 (padded; dummy cols zeroed)
                ft = ftp.tile([CD, NP], F16, tag="ft")
                nc.scalar.copy(ft[:, 0:N], acc_prev[:, :])
                nc.vector.memset(ft[:, N:NP], 0.0)
                # RF[(s,i), g] = feat[SPG*g+s, i]: SPG accumulating matmuls
                # with block one-hot stationaries (zero rows elsewhere)
                rfp = prf.tile([SPG * CD, NG], F32, tag="prf")
                for s in range(SPG):
                    nc.tensor.matmul(
                        rfp[:, :],
                        ph[0:CD, L.rsel + s * SPG * CD:L.rsel + (s + 1) * SPG * CD],
                        ft[:, s:NP:SPG],
                        start=(s == 0), stop=(s == SPG - 1),
                    )
                rf = ftp.tile([SPG * CD, NG], F16, tag="rf")
                nc.scalar.copy(rf[:, :], rfp[:, :])
                G = emit_G(c, rf[:, :], SPG * CD)
                # last cloud: only the pooled sum-of-squares is needed, and
                # it is additive over atoms -- each core of the redundant
                # pair covers half the n-columns (odd cores get an atom-
                # rotated input ordering, so the fixed [0:HN] slice lands on
                # the other global half; the host adds the two partials)
                wn = HN if c == NCLOUD - 1 else N
                acc = pacc.tile([CD, N], F32, tag="acc")
                for g in range(NG):
                    nc.tensor.matmul(
                        acc[:, 0:wn], G[:, g:CD * NG:NG],
                        hneg[:, g * N:g * N + wn],
                        start=(g == 0), stop=(g == NG - 1),
                    )
                sq = mp.tile([CD, N], F32, tag="sq")
                nc.scalar.activation(sq[:, 0:wn], acc[:, 0:wn], AF.Square,
                                     accum_out=out_sb[:, c:c + 1])
                acc_prev = acc

            nc.sync.dma_start(out=sumsq[:], in_=out_sb[:])
    return nc


_PROG_CACHE = {}


def _force_act_tables(nc):
    """Pin the ACT table chooser to the single set covering Sqrt/Abs/Square/
    Copy so no mid-kernel ACT_TABLE_LOADs are inserted."""
    import bass_rust as _bass_rust
    from concourse.hw_specs import get_activation_tables

    allowed = {"abs_reciprocal_sqrt_and_small"}
    tables = [
        (name, (funcs if name in allowed else set()))
        for name, funcs in get_activation_tables(nc.m.arch).items()
    ]

    def _patched():
        has_act = any(
            isinstance(i, mybir.InstActivation)
            for b in nc.main_func.blocks
            for i in b.instructions
        )
        if has_act:
            _bass_rust.insert_act_table_loads(nc, tables)

    nc.insert_act_table_loads = _patched


def _get_program():
    key = "v13"
    if key not in _PROG_CACHE:
        nc = bacc.Bacc(
            "TRN2", target_bir_lowering=False, debug=False,
            num_devices=NCORES,
        )
        _build(nc)
        _force_act_tables(nc)
        nc.compile()
        _PROG_CACHE[key] = nc
    return _PROG_CACHE[key]


def _f32(x):
    return np.ascontiguousarray(np.asarray(x), dtype=np.float32)


def _sp64(x):
    return np.where(x > 8.0, x, np.log1p(np.exp(np.minimum(BETA * x, 500.0))) / BETA)


def _mlp_at_r(rj, c, rad_W0, rad_W1, rad_W2, rad_Wout0, rad_Wout12):
    u = (np.asarray(rj)[:, None] - np.asarray(RADII)) / RSTEP
    basis = np.where(np.abs(u) < 1.0, np.cos(0.5 * np.pi * u) ** 2, 0.0)
    wouts = (np.asarray(rad_Wout0, np.float64),
             np.asarray(rad_Wout12[0], np.float64),
             np.asarray(rad_Wout12[1], np.float64))
    x = basis
    for Wl in (np.asarray(rad_W0[c], np.float64),
               np.asarray(rad_W1[c], np.float64),
               np.asarray(rad_W2[c], np.float64)):
        x = _sp64(x @ Wl.T / math.sqrt(Wl.shape[1]))
    return x @ wouts[c].T / math.sqrt(H)


def _tab_tables(rad_W0, rad_W1, rad_W2, rad_Wout0, rad_Wout12):
    """T[c][j, o, i] = lstsq-fitted hat-node values of MLP_c(r)/sqrt(cin)."""
    rf = np.linspace(0.0, MAXR, 4096)
    Phi = np.maximum(0.0, 1.0 - np.abs(rf[:, None] / DELTA - np.arange(J)[None, :]))
    Ts = []
    for c in range(NCLOUD):
        cin = EMB if c == 0 else CD
        Kf = _mlp_at_r(rf, c, rad_W0, rad_W1, rad_W2, rad_Wout0, rad_Wout12)
        Tl, *_ = np.linalg.lstsq(Phi, Kf, rcond=None)
        Ts.append(Tl.reshape(J, CD, cin) / math.sqrt(cin))
    return Ts


def _host_inputs(xyz, Z, emb_W, rad_W0, rad_W1, rad_W2, rad_Wout0, rad_Wout12):
    L = _PackLayout()
    xyz = _f32(xyz)
    Z = np.asarray(Z)
    Ts = _tab_tables(rad_W0, rad_W1, rad_W2, rad_Wout0, rad_Wout12)

    packh_shared = np.zeros((96, L.cols_h), np.float16)
    for c in range(NCLOUD):
        cin = EMB if c == 0 else CD
        kin = SPG * cin
        # wg[(s,i), (o, 32s'+j)] = -delta_ss' T[c][j, o, i]
        wg = np.zeros((kin, CD, SPG, J), np.float64)
        for s in range(SPG):
            # rows s*cin + i
            wg[s * cin:(s + 1) * cin, :, s, :] = -Ts[c].transpose(2, 1, 0)
        packh_shared[0:kin, L.wg[c]:L.wg[c] + CD * QP] = \
            wg.reshape(kin, CD * QP).astype(np.float16)
    # rsel[s]: [CD, 3*CD] block one-hot: col (s', i) = delta_ss' delta_ki
    for s in range(SPG):
        blk = np.zeros((CD, SPG * CD), np.float32)
        blk[:, s * CD:(s + 1) * CD] = np.eye(CD)
        packh_shared[0:CD, L.rsel + s * SPG * CD:L.rsel + (s + 1) * SPG * CD] = \
            blk.astype(np.float16)

    emb = _f32(emb_W)
    in_maps = []
    for core in range(NCORES):
        b = core // 2
        # odd cores: rotate the atom order by HN so their last-cloud [0:HN]
        # slice covers the other half of the atoms (pure relabeling;
        # clouds 0/1 sum over all atoms and are permutation-invariant)
        perm = (np.arange(N) + (HN if core % 2 else 0)) % N
        x = xyz[b][perm]
        sq = (x * x).sum(-1)
        packh = packh_shared.copy()
        # cloud-0 RF[(s,i), g] = emb[Z[SPG*g+s], i] (dummies -> 0)
        f0 = np.zeros((NP, EMB), np.float32)
        f0[0:N] = emb[Z[b][perm]]
        rf0 = f0.reshape(NG, SPG, EMB).transpose(1, 2, 0).reshape(SPG * EMB, NG)
        packh[0:SPG * EMB, L.rf0:L.rf0 + NG] = rf0.astype(np.float16)
        packf = np.zeros((128, L.cols_f), np.float32)
        onesN = np.ones(N, np.float32)
        A = np.zeros((5, NP), np.float32)
        A[0:3, 0:N] = -2 * x.T
        A[3, :] = 1.0
        A[4, 0:N] = sq
        A[4, N:NP] = 1e6                       # dummy atoms: far away (masked)
        Bm = np.stack([x[:, 0], x[:, 1], x[:, 2], sq, onesN])
        packf[0:5, L.geomA:L.geomA + NP] = A
        packf[0:5, L.geomB:L.geomB + N] = Bm
        packf[0:QP, L.absb] = -(np.arange(QP, dtype=np.float32) % J)
        in_maps.append({"packh": packh, "packf": packf})
    return in_maps


def run_device(xyz, Z, emb_W, rad_W0, rad_W1, rad_W2, rad_Wout0, rad_Wout12,
               use_collective=False, trace=False, trace_cores=None, rdt=F32R):
    """Run the device part; returns (sumsq [B, 3, CD], BassKernelResults)."""
    nc = _get_program()
    in_maps = _host_inputs(xyz, Z, emb_W, rad_W0, rad_W1, rad_W2,
                           rad_Wout0, rad_Wout12)
    res = run_bass_kernel_spmd(
        nc, in_maps, list(range(NCORES)), trace=trace,
        trace_cores=trace_cores,
    )
    out = []
    for b in range(B):
        e = res.results[2 * b]["sumsq"].copy()
        o = res.results[2 * b + 1]["sumsq"]
        e[:, NCLOUD - 1] += o[:, NCLOUD - 1]   # disjoint n-halves
        out.append(e.T)
    sumsq = np.stack(out)
    return sumsq, res


def _head(sumsq, W1, b1, g1, be1, W2, b2, g2, be2):
    x = np.sqrt(sumsq.reshape(B, NCLOUD * CD)).astype(np.float32)

    def bn(y, g, be):
        m = y.mean(0)
        v = y.var(0)
        return (y - m) / np.sqrt(v + 1e-5) * g + be

    def lrelu(y):
        return np.where(y > 0, y, 0.2 * y).astype(np.float32)

    x = lrelu(bn(x @ _f32(W1).T + _f32(b1), _f32(g1), _f32(be1)))
    x = lrelu(bn(x @ _f32(W2).T + _f32(b2), _f32(g2), _f32(be2)))
    return x.astype(np.float32)


def kernel(xyz, Z, emb_W, rad_W0, rad_W1, rad_W2, rad_Wout0, rad_Wout12,
           W1, b1, g1, be1, W2, b2, g2, be2):
    sumsq, _ = run_device(xyz, Z, emb_W, rad_W0, rad_W1, rad_W2,
                          rad_Wout0, rad_Wout12)
    return _head(sumsq, W1, b1, g1, be1, W2, b2, g2, be2)


# revision 31
# speedup vs baseline: 1.1312x; 1.1312x over previous
"""Trainium2 Bass kernel for the se3ACN encoder (gnn_message_passing).

Strategy (final: radial-MLP tabulation, J=16 nodes, 8 source atoms per matmul)
------------------------------------------------------------------------------
The per-pair radial MLP (3 -> 150 -> 150 -> 150 -> Cout*Cin, softplus) depends
only on the scalar pair distance r.  Tabulate K_c(r) = MLP_c(r)/sqrt(cin) on
J=16 piecewise-linear hats  hat_j(v) = relu(1 - |v - j|),  v = r/DELTA, with
node values least-squares fitted on a fine grid (end-to-end rel err ~1.9e-3
on hardware vs the 2e-2 gate).  The neighbor cutoff is exact: masked pairs
(r >= 3) get v shifted by +4 so every hat is exactly 0.

Per cloud the message passing becomes

    feat'[n,o] = sum_m sum_j hat[j,(m,n)] * G[m,j,o],
    G[m,j,o]   = sum_i T_c[j,o,i] * feat[m,i].

With J=16, EIGHT source atoms (group g: m = 8g+s) stack on the 128 matmul
partitions (rows q = 16*s + j), so each K=128 fp16 matmul accumulates eight
atoms' messages: 36 matmuls per cloud.  The atom count is padded 286 -> 288
with two far-away (masked) dummy atoms.

hat generation per group: a broadcast SBUF->SBUF DMA (stride-0 src AP,
issued alternately from the idle gpsimd/sync queues) replicates the eight
v rows over the eight 16-partition blocks -- no PE work; then ONE ACT Abs
(per-partition bias -(q%16)) and ONE DVE tensor_scalar (min(a,1)-1 = -hat,
sign folded into the tables) cover ABATCH groups per instruction (engine
cost is free-dim bound).  hneg stays SBUF-resident ([128, 36*286] fp16,
~21KB/partition) and serves all 3 clouds; cloud-0 accumulation is fused
into the generation loop.

Geometry: r^2 via one 5-row matmul per 96-atom chunk (the -2x.y + |x|^2 +
|y|^2 trick), then v = r/DELTA with an Abs_reciprocal_sqrt table + one
Newton step (table error squared away).  Per cloud, features are regrouped
into RF[(s,i), g] = feat[8g+s, i] by 8 accumulating matmuls with block
one-hot stationaries, then 8 table matmuls -> G (one fused psum->fp16
copy).  All ACT functions used (Abs_reciprocal_sqrt, Abs, Square, Copy)
live in the single 'abs_reciprocal_sqrt_and_small' table set -> one table
load total.

Sharding: cores (2b, 2b+1) both compute molecule b (redundant pair; the
even core's result is read back).  The 4x24 head (batch-coupled batchnorm
over the 4 molecules) runs on host.  ~72us on hardware vs the 6113us
baseline (~85x).
"""

import math

import numpy as np

import concourse.bass as bass
import concourse.mybir as mybir
import concourse.tile as tile
from concourse import bacc
from concourse.bass_utils import run_bass_kernel_spmd

AF = mybir.ActivationFunctionType
ALU = mybir.AluOpType
F32 = mybir.dt.float32
F32R = mybir.dt.float32r
F16 = mybir.dt.float16

B, N = 4, 286
EMB, CD, NCLOUD = 4, 8, 3
H = 150
BETA = 5.0
RADII = (0.0, 1.5, 3.0)
RSTEP = 1.5
MAXR = 3.0
NCORES = 8
J = 16                       # tabulation nodes
DELTA = MAXR / (J - 1)
VSHIFT = 4.0                 # pushes masked pairs out of every hat support
SPG = 8                      # source atoms per group
NP = 288                     # padded atom count (2 masked dummies)
NG = NP // SPG               # 72 groups
QP = SPG * J                 # 128 hat partitions
ABATCH = 4                   # phase-A groups emitted per stage
GEO_CHUNKS = ((0, 96), (96, 96), (192, 96))
HN = N // 2                  # last-cloud n-columns per core of a pair


class _PackLayout:
    """Column layouts of the packed constant tensors ([128, cols])."""

    def __init__(self):
        # float16 pack (staircase + tables + regroup + initial features)
        o = 0
        self.msel = o; o += 96 * J + QP             # staircase selector
        self.wg = []                                # per cloud: [SPG*cin, CD*QP]
        for c in range(NCLOUD):
            self.wg.append(o); o += CD * QP
        self.rsel = o; o += SPG * SPG * CD          # [CD, SPG*CD] per s block
        self.rf0 = o; o += NG                       # [SPG*EMB, NG] cloud-0 RF
        self.cols_h = o
        # float32 pack (geometry + abs bias)
        o = 0
        self.geomA = o; o += NP
        self.geomB = o; o += N
        self.absb = o; o += 1                       # [96, 1] = -(q % 32)
        self.cols_f = o


def _build(nc):
    L = _PackLayout()

    packh = nc.declare_dram_parameter("packh", [96, L.cols_h], F16, isOutput=False)
    packf = nc.declare_dram_parameter("packf", [128, L.cols_f], F32, isOutput=False)
    sumsq = nc.declare_dram_parameter("sumsq", [CD, NCLOUD], F32, isOutput=True)

    with tile.TileContext(nc) as tc:
        with (
            tc.tile_pool(name="const", bufs=1) as cp,
            tc.tile_pool(name="abuf", bufs=2) as ab,
            tc.tile_pool(name="gbuf", bufs=2) as gp,
            tc.tile_pool(name="ft", bufs=2) as ftp,
            tc.tile_pool(name="misc", bufs=2) as mp,
            tc.tile_pool(name="vrep", bufs=3) as vrp,
            tc.tile_pool(name="pa", bufs=2, space=bass.MemorySpace.PSUM) as pa,
            tc.tile_pool(name="pg", bufs=1, space=bass.MemorySpace.PSUM) as pg,
            tc.tile_pool(name="prf", bufs=1, space=bass.MemorySpace.PSUM) as prf,
            tc.tile_pool(name="pacc", bufs=2, space=bass.MemorySpace.PSUM) as pacc,
        ):
            pf = cp.tile([128, L.cols_f], F32, tag="packf")
            nc.sync.dma_start(out=pf[:], in_=packf[:])
            ph = cp.tile([96, L.cols_h], F16, tag="packh")
            # split the pack DMA over column slices (parallel queues), and
            # issue from gpsimd whose DMA issue cost is ~25ns vs sync ~600ns
            NSL = 8
            slw = -(-L.cols_h // NSL)
            for si in range(NSL):
                c0, c1 = si * slw, min((si + 1) * slw, L.cols_h)
                if c0 < c1:
                    nc.gpsimd.dma_start(out=ph[:, c0:c1], in_=packh[:, c0:c1])

            geomA_sb = pf[0:5, L.geomA:L.geomA + NP]
            geomB_sb = pf[0:5, L.geomB:L.geomB + N]
            absb_sb = pf[0:QP, L.absb:L.absb + 1]
            out_sb = cp.tile([CD, NCLOUD], F32, tag="out")

            # big SBUF-resident -hat matrix: [32*s + j, g*N + n] fp16
            hneg = cp.tile([QP, NG * N], F16, tag="hneg")

            # ---- geometry: v = r/DELTA (+4 where r >= MAXR), [m-chunk, N]
            v_tiles = []
            for ci, (off, pm) in enumerate(GEO_CHUNKS):
                r2p = pa.tile([128, N], F32, tag="pa")
                nc.tensor.matmul(
                    r2p[0:pm, :], geomA_sb[:, off:off + pm], geomB_sb,
                    start=True, stop=True,
                )
                r2c = mp.tile([128, N], F32, tag="r2c")
                nc.vector.tensor_scalar_max(r2c[0:pm, :], r2p[0:pm, :], 1e-12)
                # v = r/DELTA via rsqrt table + one Newton step:
                # y1 = y0*(1.5 - 0.5*r2*y0^2), r = r2*y1 (table err squared)
                y0 = mp.tile([128, N], F32, tag="y0")
                nc.scalar.activation(y0[0:pm, :], r2c[0:pm, :],
                                     AF.Abs_reciprocal_sqrt)
                y2 = mp.tile([128, N], F32, tag="y2")
                nc.scalar.activation(y2[0:pm, :], y0[0:pm, :], AF.Square)
                h2 = mp.tile([128, N], F32, tag="h2")
                nc.vector.tensor_mul(h2[0:pm, :], r2c[0:pm, :], y2[0:pm, :])
                cf = mp.tile([128, N], F32, tag="cf")
                nc.vector.tensor_scalar(
                    out=cf[0:pm, :], in0=h2[0:pm, :],
                    scalar1=float(-0.5 / DELTA), scalar2=float(1.5 / DELTA),
                    op0=ALU.mult, op1=ALU.add,
                )
                y1 = mp.tile([128, N], F32, tag="y1")
                nc.vector.tensor_mul(y1[0:pm, :], y0[0:pm, :], cf[0:pm, :])
                vt = mp.tile([128, N], F32, tag="vt")
                nc.vector.tensor_mul(vt[0:pm, :], r2c[0:pm, :], y1[0:pm, :])
                sh = mp.tile([128, N], F32, tag="sh")
                nc.vector.tensor_scalar(
                    out=sh[0:pm, :], in0=vt[0:pm, :],
                    scalar1=float(J - 1), scalar2=VSHIFT,
                    op0=ALU.is_ge, op1=ALU.mult,
                )
                vch = cp.tile([128, N], F16, tag=f"v_{ci}")
                nc.vector.tensor_add(vch[0:pm, :], vt[0:pm, :], sh[0:pm, :])
                v_tiles.append(vch)

            def emit_G(c, rf_sb, kin):
                """G[J*s+j, o*NG+g] = -sum_i T_c[j,o,i] feat[SPG*g+s,i]."""
                G = gp.tile([QP, CD * NG], F16, tag="G")
                gt = pg.tile([QP, CD * NG], F32, tag="pg")
                for o in range(CD):
                    wg_o = ph[0:kin, L.wg[c] + o * QP:L.wg[c] + (o + 1) * QP]
                    nc.tensor.matmul(gt[:, o * NG:(o + 1) * NG], wg_o, rf_sb,
                                     start=True, stop=True)
                nc.scalar.copy(G[:, :], gt[:, :])
                return G

            rf0_sb = ph[0:SPG * EMB, L.rf0:L.rf0 + NG]
            G0 = emit_G(0, rf0_sb, SPG * EMB)

            # ---- phase A: hat generation fused with cloud-0 accumulation,
            # emitted in batches so the PE sees long gapless matmul stretches
            acc0 = pacc.tile([CD, N], F32, tag="acc")
            dma_eng = [nc.gpsimd, nc.sync]
            NSEL = 12     # leading groups produced by PE selector matmuls
            batches = [[g] for g in range(NSEL)]
            g0 = NSEL
            while g0 < NG:
                batches.append(list(range(g0, min(g0 + ABATCH, NG))))
                g0 += ABATCH
            for bi, gs in enumerate(batches):
                g0 = gs[0]
                nb = len(gs)
                if nb == 1 and g0 < NSEL:
                    # PE-selector path: the staircase stationary replicates
                    # the 8 v rows over the partition blocks while the PE
                    # would otherwise idle waiting for the DMA pipeline
                    g = g0
                    ci = (SPG * g) // 96
                    off, pm = GEO_CHUNKS[ci]
                    c0 = SPG * g - off
                    u0 = pa.tile([128, N], F32, tag="pa")
                    sel = ph[0:pm, L.msel + J * c0:L.msel + J * c0 + QP]
                    nc.tensor.matmul(u0[0:QP, :], sel, v_tiles[ci][0:pm, :],
                                     start=True, stop=True)
                    a = ab.tile([QP, ABATCH * N], F16, tag="a")
                    nc.scalar.activation(a[:, 0:N], u0[0:QP, :], AF.Abs,
                                         bias=absb_sb)
                else:
                    # DMA-replication path (broadcast SBUF->SBUF, issues
                    # alternating over the idle gpsimd/sync queues)
                    vr = vrp.tile([128, ABATCH * N], F16, tag="vr")
                    for k, g in enumerate(gs):
                        ci = (SPG * g) // 96
                        off, pm = GEO_CHUNKS[ci]
                        c0 = SPG * g - off
                        dma_eng[(bi + k) % len(dma_eng)].dma_start(
                            out=vr[:, k * N:(k + 1) * N],
                            in_=v_tiles[ci][c0:c0 + SPG, :]
                            .unsqueeze(1).broadcast_to([SPG, J, N]),
                        )
                    a = ab.tile([QP, ABATCH * N], F16, tag="a")
                    nc.scalar.activation(a[:, 0:nb * N], vr[0:QP, 0:nb * N],
                                         AF.Abs, bias=absb_sb)
                nc.vector.tensor_scalar(
                    out=hneg[:, g0 * N:(g0 + nb) * N], in0=a[:, 0:nb * N],
                    scalar1=1.0, scalar2=1.0,
                    op0=ALU.min, op1=ALU.subtract,
                )
                for g in gs:
                    nc.tensor.matmul(
                        acc0[:, :], G0[:, g:CD * NG:NG],
                        hneg[:, g * N:(g + 1) * N],
                        start=(g == 0), stop=(g == NG - 1),
                    )

            # ---- cloud epilogues + clouds 1, 2
            sq = mp.tile([CD, N], F32, tag="sq")
            nc.scalar.activation(sq[:, :], acc0[:, :], AF.Square,
                                 accum_out=out_sb[:, 0:1])

            acc_prev = acc0
            for c in range(1, NCLOUD):
                # ft [8, 288] fp16 (padded; dummy cols zeroed)
                ft = ftp.tile([CD, NP], F16, tag="ft")
                nc.scalar.copy(ft[:, 0:N], acc_prev[:, :])
                nc.vector.memset(ft[:, N:NP], 0.0)
                # RF[(s,i), g] = feat[SPG*g+s, i]: SPG accumulating matmuls
                # with block one-hot stationaries (zero rows elsewhere)
                rfp = prf.tile([SPG * CD, NG], F32, tag="prf")
                for s in range(SPG):
                    nc.tensor.matmul(
                        rfp[:, :],
                        ph[0:CD, L.rsel + s * SPG * CD:L.rsel + (s + 1) * SPG * CD],
                        ft[:, s:NP:SPG],
                        start=(s == 0), stop=(s == SPG - 1),
                    )
                rf = ftp.tile([SPG * CD, NG], F16, tag="rf")
                nc.scalar.copy(rf[:, :], rfp[:, :])
                G = emit_G(c, rf[:, :], SPG * CD)
                # last cloud: only the pooled sum-of-squares is needed, and
                # it is additive over atoms -- each core of the redundant
                # pair covers half the n-columns (odd cores get an atom-
                # rotated input ordering, so the fixed [0:HN] slice lands on
                # the other global half; the host adds the two partials)
                wn = HN if c == NCLOUD - 1 else N
                acc = pacc.tile([CD, N], F32, tag="acc")
                for g in range(NG):
                    nc.tensor.matmul(
                        acc[:, 0:wn], G[:, g:CD * NG:NG],
                        hneg[:, g * N:g * N + wn],
                        start=(g == 0), stop=(g == NG - 1),
                    )
                sq = mp.tile([CD, N], F32, tag="sq")
                nc.scalar.activation(sq[:, 0:wn], acc[:, 0:wn], AF.Square,
                                     accum_out=out_sb[:, c:c + 1])
                acc_prev = acc

            nc.sync.dma_start(out=sumsq[:], in_=out_sb[:])
    return nc


_PROG_CACHE = {}


def _force_act_tables(nc):
    """Pin the ACT table chooser to the single set covering Sqrt/Abs/Square/
    Copy so no mid-kernel ACT_TABLE_LOADs are inserted."""
    import bass_rust as _bass_rust
    from concourse.hw_specs import get_activation_tables

    allowed = {"abs_reciprocal_sqrt_and_small"}
    tables = [
        (name, (funcs if name in allowed else set()))
        for name, funcs in get_activation_tables(nc.m.arch).items()
    ]

    def _patched():
        has_act = any(
            isinstance(i, mybir.InstActivation)
            for b in nc.main_func.blocks
            for i in b.instructions
        )
        if has_act:
            _bass_rust.insert_act_table_loads(nc, tables)

    nc.insert_act_table_loads = _patched


def _get_program():
    key = "v14"
    if key not in _PROG_CACHE:
        nc = bacc.Bacc(
            "TRN2", target_bir_lowering=False, debug=False,
            num_devices=NCORES,
        )
        _build(nc)
        _force_act_tables(nc)
        nc.compile()
        _PROG_CACHE[key] = nc
    return _PROG_CACHE[key]


def _f32(x):
    return np.ascontiguousarray(np.asarray(x), dtype=np.float32)


def _sp64(x):
    return np.where(x > 8.0, x, np.log1p(np.exp(np.minimum(BETA * x, 500.0))) / BETA)


def _mlp_at_r(rj, c, rad_W0, rad_W1, rad_W2, rad_Wout0, rad_Wout12):
    u = (np.asarray(rj)[:, None] - np.asarray(RADII)) / RSTEP
    basis = np.where(np.abs(u) < 1.0, np.cos(0.5 * np.pi * u) ** 2, 0.0)
    wouts = (np.asarray(rad_Wout0, np.float64),
             np.asarray(rad_Wout12[0], np.float64),
             np.asarray(rad_Wout12[1], np.float64))
    x = basis
    for Wl in (np.asarray(rad_W0[c], np.float64),
               np.asarray(rad_W1[c], np.float64),
               np.asarray(rad_W2[c], np.float64)):
        x = _sp64(x @ Wl.T / math.sqrt(Wl.shape[1]))
    return x @ wouts[c].T / math.sqrt(H)


def _tab_tables(rad_W0, rad_W1, rad_W2, rad_Wout0, rad_Wout12):
    """T[c][j, o, i] = lstsq-fitted hat-node values of MLP_c(r)/sqrt(cin)."""
    rf = np.linspace(0.0, MAXR, 4096)
    Phi = np.maximum(0.0, 1.0 - np.abs(rf[:, None] / DELTA - np.arange(J)[None, :]))
    Ts = []
    for c in range(NCLOUD):
        cin = EMB if c == 0 else CD
        Kf = _mlp_at_r(rf, c, rad_W0, rad_W1, rad_W2, rad_Wout0, rad_Wout12)
        Tl, *_ = np.linalg.lstsq(Phi, Kf, rcond=None)
        Ts.append(Tl.reshape(J, CD, cin) / math.sqrt(cin))
    return Ts


def _host_inputs(xyz, Z, emb_W, rad_W0, rad_W1, rad_W2, rad_Wout0, rad_Wout12):
    L = _PackLayout()
    xyz = _f32(xyz)
    Z = np.asarray(Z)
    Ts = _tab_tables(rad_W0, rad_W1, rad_W2, rad_Wout0, rad_Wout12)

    packh_shared = np.zeros((96, L.cols_h), np.float16)
    # staircase selector M[k, c] = [k == c//J]
    ncols = 96 * J + QP
    cols = np.arange(ncols) // J
    packh_shared[:, L.msel:L.msel + ncols] = (
        np.arange(96)[:, None] == cols[None, :]).astype(np.float16)
    for c in range(NCLOUD):
        cin = EMB if c == 0 else CD
        kin = SPG * cin
        # wg[(s,i), (o, 32s'+j)] = -delta_ss' T[c][j, o, i]
        wg = np.zeros((kin, CD, SPG, J), np.float64)
        for s in range(SPG):
            # rows s*cin + i
            wg[s * cin:(s + 1) * cin, :, s, :] = -Ts[c].transpose(2, 1, 0)
        packh_shared[0:kin, L.wg[c]:L.wg[c] + CD * QP] = \
            wg.reshape(kin, CD * QP).astype(np.float16)
    # rsel[s]: [CD, 3*CD] block one-hot: col (s', i) = delta_ss' delta_ki
    for s in range(SPG):
        blk = np.zeros((CD, SPG * CD), np.float32)
        blk[:, s * CD:(s + 1) * CD] = np.eye(CD)
        packh_shared[0:CD, L.rsel + s * SPG * CD:L.rsel + (s + 1) * SPG * CD] = \
            blk.astype(np.float16)

    emb = _f32(emb_W)
    in_maps = []
    for core in range(NCORES):
        b = core // 2
        # odd cores: rotate the atom order by HN so their last-cloud [0:HN]
        # slice covers the other half of the atoms (pure relabeling;
        # clouds 0/1 sum over all atoms and are permutation-invariant)
        perm = (np.arange(N) + (HN if core % 2 else 0)) % N
        x = xyz[b][perm]
        sq = (x * x).sum(-1)
        packh = packh_shared.copy()
        # cloud-0 RF[(s,i), g] = emb[Z[SPG*g+s], i] (dummies -> 0)
        f0 = np.zeros((NP, EMB), np.float32)
        f0[0:N] = emb[Z[b][perm]]
        rf0 = f0.reshape(NG, SPG, EMB).transpose(1, 2, 0).reshape(SPG * EMB, NG)
        packh[0:SPG * EMB, L.rf0:L.rf0 + NG] = rf0.astype(np.float16)
        packf = np.zeros((128, L.cols_f), np.float32)
        onesN = np.ones(N, np.float32)
        A = np.zeros((5, NP), np.float32)
        A[0:3, 0:N] = -2 * x.T
        A[3, :] = 1.0
        A[4, 0:N] = sq
        A[4, N:NP] = 1e6                       # dummy atoms: far away (masked)
        Bm = np.stack([x[:, 0], x[:, 1], x[:, 2], sq, onesN])
        packf[0:5, L.geomA:L.geomA + NP] = A
        packf[0:5, L.geomB:L.geomB + N] = Bm
        packf[0:QP, L.absb] = -(np.arange(QP, dtype=np.float32) % J)
        in_maps.append({"packh": packh, "packf": packf})
    return in_maps


def run_device(xyz, Z, emb_W, rad_W0, rad_W1, rad_W2, rad_Wout0, rad_Wout12,
               use_collective=False, trace=False, trace_cores=None, rdt=F32R):
    """Run the device part; returns (sumsq [B, 3, CD], BassKernelResults)."""
    nc = _get_program()
    in_maps = _host_inputs(xyz, Z, emb_W, rad_W0, rad_W1, rad_W2,
                           rad_Wout0, rad_Wout12)
    res = run_bass_kernel_spmd(
        nc, in_maps, list(range(NCORES)), trace=trace,
        trace_cores=trace_cores,
    )
    out = []
    for b in range(B):
        e = res.results[2 * b]["sumsq"].copy()
        o = res.results[2 * b + 1]["sumsq"]
        e[:, NCLOUD - 1] += o[:, NCLOUD - 1]   # disjoint n-halves
        out.append(e.T)
    sumsq = np.stack(out)
    return sumsq, res


def _head(sumsq, W1, b1, g1, be1, W2, b2, g2, be2):
    x = np.sqrt(sumsq.reshape(B, NCLOUD * CD)).astype(np.float32)

    def bn(y, g, be):
        m = y.mean(0)
        v = y.var(0)
        return (y - m) / np.sqrt(v + 1e-5) * g + be

    def lrelu(y):
        return np.where(y > 0, y, 0.2 * y).astype(np.float32)

    x = lrelu(bn(x @ _f32(W1).T + _f32(b1), _f32(g1), _f32(be1)))
    x = lrelu(bn(x @ _f32(W2).T + _f32(b2), _f32(g2), _f32(be2)))
    return x.astype(np.float32)


def kernel(xyz, Z, emb_W, rad_W0, rad_W1, rad_W2, rad_Wout0, rad_Wout12,
           W1, b1, g1, be1, W2, b2, g2, be2):
    sumsq, _ = run_device(xyz, Z, emb_W, rad_W0, rad_W1, rad_W2,
                          rad_Wout0, rad_Wout12)
    return _head(sumsq, W1, b1, g1, be1, W2, b2, g2, be2)
